# revision 1
# baseline (speedup 1.0000x reference)
"""Distributed Trainium2 Bass kernel for BrosAttention.

B=2, S=1024, H=768, NH=12, DH=64:
  q,k,v = heads(hidden @ W.T + b)
  scores = q@k^T + einsum('bnid,bijd->bnij', q, bpe)   (bpe = bbox transposed)
  probs  = softmax(scores / 8)
  out    = LN(probs@v @ Wo.T + bo + hidden)

Sharding: 8 cores = 2 batches x 4 query-row blocks of 256 rows. Each core
reads only its 64MB slice of bbox_pos_emb, computes K/V for the full
sequence of its batch (duplicated 4x, cheaper than a collective here), and
writes a disjoint [256, 768] output slice. No collectives.

Layout: transposed scores (scoresT[j, i] per head) because the bias term
q.bpe needs d on partitions; bpe arrives [j, d] and is PE-transposed with
two query rows packed per [128, j] tile. The bias matmul (lhsT = q of one
row as a [64, 12] weight) runs 4 i's concurrently in the four 32-column
groups of the PE array; bias tiles are PE-transposed again into [j, (i,n)]
and added to QK^T psum tiles via a stride-12 AP. Softmax-over-partitions
uses ones-vector matmuls; probs stay unnormalized until after P@V.
"""

import os
import sys
import numpy as np

sys.path.insert(0, "/opt/trn_rl_repo")

B, S, H, NH, DH = 2, 1024, 768, 12, 64
EPS = 1e-12
P = 128
I_CORE = S * B // 8  # 256
N_CORES = 8

_COMPILED = {}


def build_kernel(s=S, i_core=I_CORE, h=H, nh=NH, dh=DH):
    from contextlib import ExitStack
    from concourse import bacc, bass, mybir, tile

    f32 = mybir.dt.float32
    bf16 = mybir.dt.bfloat16
    Alu = mybir.AluOpType
    Act = mybir.ActivationFunctionType
    AxisX = mybir.AxisListType.X

    SC = s // P          # 8 seq chunks
    HC = h // P          # 6 hidden chunks
    IH = i_core // 2     # 128 i's per half
    NDUO_H = IH // 4     # 32 duos per half
    JH = min(512, s)     # fp32 matmul N limit / psum bank
    NJH = s // JH        # 2
    HP = nh // 2         # 6 head pairs
    VH = h // 2          # 384

    nc = bacc.Bacc(None, target_bir_lowering=False, debug=False)

    bf16_ = mybir.dt.bfloat16
    d_hidT = nc.declare_dram_parameter("hidT", [HC, P, s], bf16_, isOutput=False)
    d_hidRT = nc.declare_dram_parameter("hidRT", [HC, P, i_core], bf16_, isOutput=False)
    d_hidR = nc.declare_dram_parameter("hid_rows", [i_core // P, P, h], f32, isOutput=False)
    d_bpe = nc.declare_dram_parameter("bpe", [i_core, dh, s], bf16_, isOutput=False)
    d_W = {w: nc.declare_dram_parameter(w + "T", [HC, P, h], bf16_, isOutput=False)
           for w in ("Wq", "Wk", "Wv", "Wo")}
    d_b = {bn: nc.declare_dram_parameter(bn, [1, h], f32, isOutput=False)
           for bn in ("bq", "bk", "bv", "bo", "ln_gamma", "ln_beta")}
    d_ident = nc.declare_dram_parameter("ident", [P, P], f32, isOutput=False)
    d_out = nc.declare_dram_parameter("out", [i_core // P, P, h], f32, isOutput=True)

    with tile.TileContext(nc) as tc, ExitStack() as ctx:
        # ------------- long-lived pools -------------
        const_p = ctx.enter_context(tc.tile_pool(name="const", bufs=1))
        stat_p = ctx.enter_context(tc.tile_pool(name="stat", bufs=1))
        ps128 = ctx.enter_context(
            tc.tile_pool(name="ps128", bufs=3, space=bass.MemorySpace.PSUM))
        ps512 = ctx.enter_context(
            tc.tile_pool(name="ps512", bufs=1, space=bass.MemorySpace.PSUM))
        psB = ctx.enter_context(
            tc.tile_pool(name="psB", bufs=2, space=bass.MemorySpace.PSUM))
        psS = ctx.enter_context(
            tc.tile_pool(name="psS", bufs=1, space=bass.MemorySpace.PSUM))
        psC = ctx.enter_context(
            tc.tile_pool(name="psC", bufs=1, space=bass.MemorySpace.PSUM))

        # ------------- constants -------------
        ident = const_p.tile([P, P], f32)
        nc.sync.dma_start(ident[:], d_ident[:])
        ones_col = const_p.tile([P, 1], f32)
        nc.vector.memset(ones_col[:], 1.0)
        ones_row = const_p.tile([1, s], f32)
        nc.vector.memset(ones_row[:], 1.0)
        eps_t = const_p.tile([P, 1], f32)
        nc.vector.memset(eps_t[:], EPS)
        zrow = const_p.tile([1, P], bf16)
        nc.vector.memset(zrow[:], 0.0)
        ident_bf = const_p.tile([P, P], bf16)
        nc.vector.tensor_copy(ident_bf[:], ident[:])
        ones_col_bf = const_p.tile([P, 1], bf16)
        nc.vector.memset(ones_col_bf[:], 1.0)
        ones_row_bf = const_p.tile([1, s], bf16)
        nc.vector.memset(ones_row_bf[:], 1.0)
        b_sb = {}
        b_bf = {}
        for bn in ("bq", "bk", "bv", "bo", "ln_gamma", "ln_beta"):
            b_sb[bn] = const_p.tile([1, h], f32, name=f"bias_{bn}")
            nc.sync.dma_start(b_sb[bn][:], d_b[bn][:])
            b_bf[bn] = const_p.tile([1, h], bf16, name=f"biasbf_{bn}")
            nc.vector.tensor_copy(b_bf[bn][:], b_sb[bn][:])

        bcast = {}
        for bn in ("ln_gamma", "ln_beta"):
            t = stat_p.tile([P, h], f32, name=f"bcast_{bn}")
            for c in range(HC):
                pbx = ps128.tile([P, P], f32, name="pt")
                nc.tensor.matmul(pbx[:], ones_row[:, 0:P],
                                 b_sb[bn][:, c * P:(c + 1) * P])
                nc.scalar.copy(t[:, c * P:(c + 1) * P], pbx[:])
            bcast[bn] = t

        # long-lived activations
        hidR = stat_p.tile([P, i_core // P, h], f32)
        nc.sync.dma_start(hidR[:], d_hidR[:].transpose([1, 0, 2]))
        WoT = stat_p.tile([P, HC, h], bf16)
        nc.sync.dma_start(WoT[:], d_W["Wo"][:].transpose([1, 0, 2]))
        qT128 = stat_p.tile([P, nh, i_core], bf16)  # q[n,i,:] at both 64-halves
        qPair = stat_p.tile([P, i_core // 2, 32], bf16)
        kT128 = stat_p.tile([P, HP, s], bf16)
        v_sb = stat_p.tile([P, SC, h], bf16)

        def pe_T(dst_ap, src_ap, copy_eng):
            bf = src_ap.dtype == bf16
            pt = ps128.tile([P, P], bf16 if bf else f32, name="pt")
            n = src_ap.shape[-1]
            nc.tensor.transpose(pt[0:n, :], src_ap,
                                ident_bf[:] if bf else ident[:])
            if copy_eng is nc.scalar:
                copy_eng.copy(dst_ap, pt[0:n, :])
            else:
                copy_eng.tensor_copy(dst_ap, pt[0:n, :])

        # ------------- phase 0 -------------
        with tc.tile_pool(name="early", bufs=1) as early_p:
            hidT = early_p.tile([P, HC, s], bf16)
            nc.sync.dma_start(hidT[:], d_hidT[:].transpose([1, 0, 2]))
            hidRT = early_p.tile([P, HC, i_core], bf16)
            nc.sync.dma_start(hidRT[:], d_hidRT[:].transpose([1, 0, 2]))

            def load_WT(w, dst):
                nc.sync.dma_start(dst[:], d_W[w][:].transpose([1, 0, 2]))
                return dst

            # Q projection (transposed): qT = Wq @ hidR^T + bq
            WqT = load_WT("Wq", early_p.tile([P, HC, h], bf16, name="WT_q"))
            for r in range(HC):
                pq = ps512.tile([P, JH], f32, name="big")
                for kc in range(HC):
                    nc.tensor.matmul(pq[:, 0:i_core],
                                     WqT[:, kc, r * P:(r + 1) * P],
                                     hidRT[:, kc, :], start=(kc == 0), stop=False)
                nc.tensor.matmul(pq[:, 0:i_core], b_bf["bq"][:, r * P:(r + 1) * P],
                                 ones_row_bf[:, 0:i_core], start=False, stop=True)
                for sub in range(2):
                    src = pq[sub * dh:(sub + 1) * dh, 0:i_core]
                    nc.vector.tensor_copy(qT128[0:dh, 2 * r + sub, :], src)
                    nc.vector.tensor_copy(qT128[dh:P, 2 * r + sub, :], src)

            # qPair[k, p, m]: block-diag bias weights: rows 0-63 =
            # q_{2p} in cols 0:12, rows 64-127 = q_{2p+1} in cols 12:24.
            nc.vector.memset(qPair[:], 0.0)
            nc.vector.tensor_copy(
                qPair[0:dh, :, 0:nh],
                qT128[0:dh, :, 0::2].transpose([0, 2, 1]))
            nc.vector.tensor_copy(
                qPair[dh:P, :, nh:2 * nh],
                qT128[dh:P, :, 1::2].transpose([0, 2, 1]))

            # K projection (transposed): kT = Wk @ hid^T + bk
            WkT = load_WT("Wk", early_p.tile([P, HC, h], bf16, name="WT_k"))
            for r in range(HC):
                for jh in range(NJH):
                    pk = ps512.tile([P, JH], f32, name="big")
                    for kc in range(HC):
                        nc.tensor.matmul(pk[:], WkT[:, kc, r * P:(r + 1) * P],
                                         hidT[:, kc, jh * JH:(jh + 1) * JH],
                                         start=(kc == 0), stop=False)
                    nc.tensor.matmul(pk[:], b_bf["bk"][:, r * P:(r + 1) * P],
                                     ones_row_bf[:, 0:JH], start=False, stop=True)
                    nc.vector.tensor_copy(
                        kT128[:, r, jh * JH:(jh + 1) * JH], pk[:])

            # V projection (natural): v = hid @ Wv^T + bv
            WvT = load_WT("Wv", early_p.tile([P, HC, h], bf16, name="WT_v"))
            for jc in range(SC):
                for vh in range(2):
                    pv = ps512.tile([P, JH], f32, name="big")
                    for kc in range(HC):
                        nc.tensor.matmul(pv[:, 0:VH],
                                         hidT[:, kc, jc * P:(jc + 1) * P],
                                         WvT[:, kc, vh * VH:(vh + 1) * VH],
                                         start=(kc == 0), stop=False)
                    nc.tensor.matmul(pv[:, 0:VH], ones_row_bf[:, 0:P],
                                     b_bf["bv"][:, vh * VH:(vh + 1) * VH],
                                     start=False, stop=True)
                    nc.vector.tensor_copy(v_sb[:, jc, vh * VH:(vh + 1) * VH],
                                          pv[:, 0:VH])


        # ------------- phases A+B -------------
        with tc.tile_pool(name="bpeT", bufs=4) as bpeT_p, \
             tc.tile_pool(name="bias4", bufs=1) as bias4_p, \
             tc.tile_pool(name="biasT", bufs=1) as biasT_p, \
             tc.tile_pool(name="sm", bufs=2) as sm_p, \
             tc.tile_pool(name="ctxp", bufs=1) as ctx_p, \
             tc.tile_pool(name="yp", bufs=1) as y_p:
            for half in range(2):
                i0h = half * IH
                # biasT[j, jc, duo*48 + 12*i4 + n]
                biasT = biasT_p.tile([P, SC, NDUO_H * 4 * nh], bf16)

                for octo in range(NDUO_H // 2):
                    pb_h = [psB.tile([P, JH], f32, name="pbh") for j in range(NJH)]
                    for c4 in range(4):
                        pair = octo * 4 + c4
                        iA = i0h + 2 * pair
                        bpeT = bpeT_p.tile([P, s], bf16)
                        nc.sync.dma_start(
                            bpeT[:], d_bpe[iA:iA + 2].rearrange("a b c -> (a b) c"))
                        lhs = qPair[:, (i0h // 2) + pair, 0:32]
                        for jh in range(NJH):
                            nc.tensor.matmul(
                                pb_h[jh][32 * c4:32 * c4 + 32, :], lhs,
                                bpeT[:, jh * JH:(jh + 1) * JH],
                                tile_position=(0, 32 * c4))
                    b4 = bias4_p.tile([P, s], bf16)
                    for jh in range(NJH):
                        nc.vector.tensor_copy(b4[:, jh * JH:(jh + 1) * JH],
                                              pb_h[jh][:])
                    for jc in range(SC):
                        ptb = ps128.tile([P, P], bf16, name="pt")
                        nc.tensor.transpose(ptb[:], b4[:, jc * P:(jc + 1) * P],
                                            ident_bf[:])
                        nc.vector.tensor_copy(
                            biasT[:, jc, octo * 8 * nh:(octo + 1) * 8 * nh
                                  ].rearrange("p (a b) -> p a b", a=4),
                            ptb[:].rearrange("p (a b) -> p a b", a=4)[:, :, 0:2 * nh])

                # ---- attention ----
                ctxT = ctx_p.tile([P, HP, IH], bf16)
                for hp in range(HP):
                    pctx = psC.tile([P, IH], f32, name="pctx")
                    for sub in range(2):
                        n = 2 * hp + sub
                        probsT = sm_p.tile([P, SC, IH], bf16)
                        psum_s = psS.tile([1, IH], f32)
                        for jc in range(SC):
                            pqk = ps128.tile([P, IH], f32, name="pt")
                            sb = sub * dh
                            nc.tensor.matmul(pqk[:],
                                             kT128[sb:sb + dh, hp, jc * P:(jc + 1) * P],
                                             qT128[sb:sb + dh, n, i0h:i0h + IH])
                            sE = sm_p.tile([P, IH], f32)
                            nc.vector.tensor_tensor(
                                sE[:], pqk[:],
                                biasT[:, jc, n:n + nh * (IH - 1) + 1:nh], Alu.add)
                            nc.scalar.activation(probsT[:, jc, :], sE[:],
                                                 Act.Exp, scale=0.125)
                            nc.tensor.matmul(psum_s[:], ones_col_bf[:],
                                             probsT[:, jc, :],
                                             start=(jc == 0), stop=(jc == SC - 1),
                                             skip_group_check=True)
                        rec = sm_p.tile([1, IH], f32)
                        nc.vector.reciprocal(rec[:], psum_s[:])
                        prec = ps128.tile([P, IH], f32, name="pt")
                        nc.tensor.matmul(prec[0:dh, :], ones_row[:, 0:dh], rec[:])
                        recB = sm_p.tile([dh, IH], f32)
                        nc.scalar.copy(recB[:], prec[0:dh, :])
                        for jc in range(SC):
                            nc.tensor.matmul(
                                pctx[sub * dh:(sub + 1) * dh, :],
                                v_sb[:, jc, n * dh:(n + 1) * dh],
                                probsT[:, jc, :],
                                start=(jc == 0), stop=(jc == SC - 1),
                                tile_position=(0, sub * dh),
                                skip_group_check=True)
                        nc.vector.tensor_tensor(
                            pctx[sub * dh:(sub + 1) * dh, :],
                            pctx[sub * dh:(sub + 1) * dh, :],
                            recB[:], Alu.mult)
                    nc.scalar.copy(ctxT[:, hp, :], pctx[:])

                # ---- O-proj + residual + LN ----
                pys = [ps512.tile([P, VH], f32, name="big") for j in range(2)]
                for vh in range(2):
                    for kc in range(HC):
                        nc.tensor.matmul(pys[vh][:], ctxT[:, kc, :],
                                         WoT[:, kc, vh * VH:(vh + 1) * VH],
                                         start=(kc == 0), stop=False)
                    nc.tensor.matmul(pys[vh][:], ones_row_bf[:, 0:P],
                                     b_bf["bo"][:, vh * VH:(vh + 1) * VH],
                                     start=False, stop=True)
                y = y_p.tile([P, h], f32)
                for vh in range(2):
                    nc.vector.tensor_tensor(y[:, vh * VH:(vh + 1) * VH],
                                            pys[vh][:],
                                            hidR[:, half, vh * VH:(vh + 1) * VH],
                                            Alu.add)
                mu = y_p.tile([P, 1], f32)
                nc.vector.tensor_reduce(mu[:], y[:], AxisX, Alu.add)
                nc.vector.tensor_scalar(mu[:], mu[:], 1.0 / h, None, Alu.mult)
                yc = y_p.tile([P, h], f32)
                nc.vector.tensor_scalar(yc[:], y[:], mu[:], None, Alu.subtract)
                ssq = y_p.tile([P, 1], f32)
                nc.scalar.activation(y[:], yc[:], Act.Square, accum_out=ssq[:])
                std = y_p.tile([P, 1], f32)
                nc.scalar.activation(std[:], ssq[:], Act.Sqrt,
                                     scale=1.0 / h, bias=eps_t[:])
                rstd = y_p.tile([P, 1], f32)
                nc.vector.reciprocal(rstd[:], std[:])
                o1 = y_p.tile([P, h], f32)
                nc.vector.tensor_scalar(o1[:], yc[:], rstd[:], None, Alu.mult)
                nc.vector.tensor_tensor(o1[:], o1[:], bcast["ln_gamma"][:], Alu.mult)
                nc.vector.tensor_tensor(o1[:], o1[:], bcast["ln_beta"][:], Alu.add)
                nc.sync.dma_start(d_out[half], o1[:])

    nc.compile()
    return nc


def _shard_inputs(inputs):
    import ml_dtypes
    bf = ml_dtypes.bfloat16
    hs = np.ascontiguousarray(np.asarray(inputs["hidden_states"]), dtype=np.float32)
    bpe = np.asarray(inputs["bbox_pos_emb"])
    ident = np.eye(P, dtype=np.float32)
    # per-batch transposed hidden [H, S] in bf16
    hsT = {b: np.ascontiguousarray(hs[b].T.astype(bf)).reshape(H // P, P, S)
           for b in range(B)}
    WT = {w: np.ascontiguousarray(
             np.asarray(inputs[w], dtype=np.float32).T.astype(bf)).reshape(
                 H // P, P, H)
          for w in ("Wq", "Wk", "Wv", "Wo")}
    in_maps = []
    for c in range(N_CORES):
        b = c // 4
        q0 = (c % 4) * I_CORE
        m = {
            "hidT": hsT[b],
            "hidRT": np.ascontiguousarray(
                hs[b, q0:q0 + I_CORE].T.astype(bf)).reshape(H // P, P, I_CORE),
            "hid_rows": np.ascontiguousarray(
                hs[b, q0:q0 + I_CORE].reshape(I_CORE // P, P, H)),
            "bpe": np.ascontiguousarray(
                bpe[q0:q0 + I_CORE, :, b, :].transpose(0, 2, 1).astype(bf)),
            "ident": ident,
        }
        for w in ("Wq", "Wk", "Wv", "Wo"):
            m[w + "T"] = WT[w]
        for bn in ("bq", "bk", "bv", "bo", "ln_gamma", "ln_beta"):
            m[bn] = np.ascontiguousarray(
                np.asarray(inputs[bn], dtype=np.float32).reshape(1, H))
        in_maps.append(m)
    return in_maps


def _install_ntff_shim():
    """The agent image's antenv lacks axon_hooks; recreate the NTFF profile
    hook via ctypes against libaxon_pjrt.so so trace=True yields
    exec_time_ns + a perfetto trace."""
    import sys as _sys
    if "antenv.axon_hooks" in _sys.modules:
        return
    import types, ctypes, contextlib
    so_path = "/opt/axon/libaxon_pjrt.so"
    mod = types.ModuleType("antenv.axon_hooks")
    _state = {}

    def get_axon_ntff_profile_hook():
        if "hook" in _state:
            return _state["hook"]
        try:
            lib = ctypes.CDLL(so_path)
            if not hasattr(lib, "axon_start_nrt_profile"):
                _state["hook"] = None
                return None
            lib.axon_start_nrt_profile.argtypes = [
                ctypes.POINTER(ctypes.c_int64), ctypes.c_size_t]
            lib.axon_start_nrt_profile.restype = ctypes.c_int64
            lib.axon_stop_nrt_profile.argtypes = [ctypes.c_char_p]
            lib.axon_stop_nrt_profile.restype = ctypes.c_int64
        except OSError:
            _state["hook"] = None
            return None

        @contextlib.contextmanager
        def _hook(output_dir, device_ids):
            import jax
            jax.devices()
            if device_ids:
                ids = (ctypes.c_int64 * len(device_ids))(*device_ids)
                rc = lib.axon_start_nrt_profile(ids, len(device_ids))
            else:
                rc = lib.axon_start_nrt_profile(None, 0)
            if rc != 0:
                raise RuntimeError(f"axon_start_nrt_profile rc={rc}")
            try:
                yield
            finally:
                n = lib.axon_stop_nrt_profile(str(output_dir).encode())
                print(f"ntff profile: {n} file(s) written to {output_dir}")

        _state["hook"] = _hook
        return _hook

    mod.get_axon_ntff_profile_hook = get_axon_ntff_profile_hook
    _sys.modules["antenv.axon_hooks"] = mod


def kernel(**inputs):
    from concourse.bass_utils import run_bass_kernel_spmd

    if os.environ.get("BASS_KERNEL_TRACE"):
        _install_ntff_shim()
        import concourse.bass_utils as _bu
        _bu.upload_artifacts = lambda tmpdir: f"file://{tmpdir}"

    if "nc" not in _COMPILED:
        _COMPILED["nc"] = build_kernel()
    nc = _COMPILED["nc"]
    in_maps = _shard_inputs(inputs)
    res = run_bass_kernel_spmd(nc, in_maps, core_ids=list(range(N_CORES)),
                               trace=bool(os.environ.get("BASS_KERNEL_TRACE")))
    _COMPILED["last_result"] = res
    out = np.zeros((B, S, H), dtype=np.float32)
    for c in range(N_CORES):
        b = c // 4
        q0 = (c % 4) * I_CORE
        out[b, q0:q0 + I_CORE] = np.asarray(
            res.results[c]["out"]).reshape(I_CORE, H)
    return out



# revision 14
# speedup vs baseline: 1.3975x; 1.3975x over previous
"""Distributed Trainium2 Bass kernel for BrosAttention (v2).

B=2, S=1024, H=768, NH=12, DH=64:
  q,k,v = heads(hidden @ W.T + b)
  scores = q@k^T + einsum('bnid,bijd->bnij', q, bpe)   (bpe = bbox transposed)
  probs  = softmax(scores / 8)
  out    = LN(probs@v @ Wo.T + bo + hidden)

Sharding: 8 cores = 2 batches x 4 query-row blocks of 256 rows. Each core
reads only its slice of bbox_pos_emb, computes K/V for the full sequence of
its batch, and writes a disjoint [256, 768] output slice. No collectives.

v2: transposed scores per head [j, i]; probs factorized as
exp(qk/8) * exp(bias/8) with both exps fused into the PSUM evacuations on
the scalar engine. Bias per octo (8 i's) is duo-packed (block-diag q
weights, 4 col groups), exp'd into E2pre, PE-transposed per j-chunk into a
2-bank PSUM tile with columns (octo, c4, i', n); a nested-AP DVE mult forms
probsT[j, n, i]. P@V uses V augmented with a ones column so each head's
softmax denominator lands in PSUM row 64; ctx is rescaled by 1/denom during
evacuation. O-proj contracts per head (K=64) from partition-0-aligned ctx.
"""

import os
import sys
import numpy as np

sys.path.insert(0, "/opt/trn_rl_repo")

B, S, H, NH, DH = 2, 1024, 768, 12, 64
EPS = 1e-12
P = 128
I_CORE = S * B // 8  # 256
N_CORES = 8

_COMPILED = {}


def build_kernel(s=S, i_core=I_CORE, h=H, nh=NH, dh=DH):
    from contextlib import ExitStack
    from concourse import bacc, bass, mybir, tile

    STAGE = int(os.environ.get("BASS_V2_STAGE", "4"))  # 1=ph0 2=+A 3=+B 4=full
    SUB = int(os.environ.get("BASS_V2_SUB", "4"))  # 1=T 2=+QK 3=+TT 4=+PV

    f32 = mybir.dt.float32
    bf16 = mybir.dt.bfloat16
    Alu = mybir.AluOpType
    Act = mybir.ActivationFunctionType
    AxisX = mybir.AxisListType.X

    SC = s // P          # 8 seq chunks
    HC = h // P          # 6 hidden chunks
    IH = i_core // 2     # 128 i's per half
    NOCT = IH // 8       # 16 octos per half
    JH = min(512, s)     # fp32 matmul N limit / psum bank
    NJH = s // JH        # 2
    HP = nh // 2         # 6 head pairs
    VH = h // 2          # 384

    nc = bacc.Bacc(None, target_bir_lowering=False, debug=False)

    d_hidT = nc.declare_dram_parameter("hidT", [HC, P, s], bf16, isOutput=False)
    d_hidRT = nc.declare_dram_parameter("hidRT", [HC, P, i_core], bf16, isOutput=False)
    d_hidR = nc.declare_dram_parameter("hid_rows", [i_core // P, P, h], f32, isOutput=False)
    d_bpe = nc.declare_dram_parameter("bpe", [i_core, dh, s], bf16, isOutput=False)
    d_W = {w: nc.declare_dram_parameter(w + "T", [HC, P, h], bf16, isOutput=False)
           for w in ("Wq", "Wk", "Wv")}
    d_WoTh = nc.declare_dram_parameter("WoTh", [dh, nh, h], bf16, isOutput=False)
    d_b = {bn: nc.declare_dram_parameter(bn, [1, h], f32, isOutput=False)
           for bn in ("bq", "bk", "bv", "bo", "ln_gamma", "ln_beta")}
    d_ident = nc.declare_dram_parameter("ident", [P, P], f32, isOutput=False)
    d_out = nc.declare_dram_parameter("out", [i_core // P, P, h], f32, isOutput=True)

    with tile.TileContext(nc) as tc, ExitStack() as ctx:
        # ------------- long-lived pools -------------
        const_p = ctx.enter_context(tc.tile_pool(name="const", bufs=1))
        stat_p = ctx.enter_context(tc.tile_pool(name="stat", bufs=1))
        bpeT_p = ctx.enter_context(tc.tile_pool(name="bpeT", bufs=6))

        # ------------- constants -------------
        ident = const_p.tile([P, P], f32)
        nc.sync.dma_start(ident[:], d_ident[:])
        ones_row = const_p.tile([1, s], f32)
        nc.vector.memset(ones_row[:], 1.0)
        eps_t = const_p.tile([P, 1], f32)
        nc.vector.memset(eps_t[:], EPS)
        ident_bf = const_p.tile([P, P], bf16)
        nc.vector.tensor_copy(ident_bf[:], ident[:])
        ones_row_bf = const_p.tile([1, s], bf16)
        nc.vector.memset(ones_row_bf[:], 1.0)
        # ones row living on partition 64 (for the denom broadcast matmul)
        ones_p64 = const_p.tile([P, dh], f32)
        nc.vector.memset(ones_p64[dh:dh + 1, :], 1.0)
        b_sb = {}
        b_bf = {}
        for bn in ("bq", "bk", "bv", "bo", "ln_gamma", "ln_beta"):
            b_sb[bn] = const_p.tile([1, h], f32, name=f"bias_{bn}")
            nc.sync.dma_start(b_sb[bn][:], d_b[bn][:])
            b_bf[bn] = const_p.tile([1, h], bf16, name=f"biasbf_{bn}")
            nc.vector.tensor_copy(b_bf[bn][:], b_sb[bn][:])

        with tc.tile_pool(name="ps_bc", bufs=2, space=bass.MemorySpace.PSUM) as ps_bc:
            bcast = {}
            for bn in ("ln_gamma", "ln_beta"):
                t = stat_p.tile([P, h], f32, name=f"bcast_{bn}")
                for c in range(HC):
                    pbx = ps_bc.tile([P, P], f32, name="pt")
                    nc.tensor.matmul(pbx[:], ones_row[:, 0:P],
                                     b_sb[bn][:, c * P:(c + 1) * P])
                    nc.scalar.copy(t[:, c * P:(c + 1) * P], pbx[:])
                bcast[bn] = t

        # long-lived activations
        hidR = stat_p.tile([P, i_core // P, h], f32)
        nc.sync.dma_start(hidR[:], d_hidR[:].transpose([1, 0, 2]))
        WoTh = stat_p.tile([dh, nh, h], bf16)
        nc.sync.dma_start(WoTh[:], d_WoTh[:])
        qT128 = stat_p.tile([P, nh, i_core], bf16)  # q[n,i,:] at both 64-halves
        qPair = stat_p.tile([P, i_core // 2, 32], bf16)
        # kTfull[d128, n, j]: head n's k^T in its own 64-half, zeros in the
        # other half -> every QK matmul is uniform K=128 at base partition 0.
        # (alternating 0/64-base K=64 matmuls drain concurrently into one
        # PSUM bank -> fatal HW collision; this layout avoids that.)
        kTfull = stat_p.tile([P, nh, s], bf16)
        nc.vector.memset(kTfull[:], 0.0)
        v_aug = stat_p.tile([P, SC, nh, dh + 1], bf16)
        nc.vector.memset(
            v_aug[:].rearrange("p a b c -> p (a b) c")[:, :, dh:dh + 1], 1.0)

        # ------------- phase 0: projections -------------
        with tc.tile_pool(name="early", bufs=1) as early_p, \
             tc.tile_pool(name="ps512", bufs=3, space=bass.MemorySpace.PSUM) as ps512:
            hidT = early_p.tile([P, HC, s], bf16)
            nc.sync.dma_start(hidT[:], d_hidT[:].transpose([1, 0, 2]))
            hidRT = early_p.tile([P, HC, i_core], bf16)
            nc.sync.dma_start(hidRT[:], d_hidRT[:].transpose([1, 0, 2]))

            def load_WT(w, dst):
                nc.sync.dma_start(dst[:], d_W[w][:].transpose([1, 0, 2]))
                return dst

            # Q projection (transposed): qT = Wq @ hidR^T + bq
            WqT = load_WT("Wq", early_p.tile([P, HC, h], bf16, name="WT_q"))
            for r in range(HC):
                pq = ps512.tile([P, JH], f32, name="big")
                for kc in range(HC):
                    nc.tensor.matmul(pq[:, 0:i_core],
                                     WqT[:, kc, r * P:(r + 1) * P],
                                     hidRT[:, kc, :], start=(kc == 0), stop=False)
                nc.tensor.matmul(pq[:, 0:i_core], b_bf["bq"][:, r * P:(r + 1) * P],
                                 ones_row_bf[:, 0:i_core], start=False, stop=True)
                for sub in range(2):
                    src = pq[sub * dh:(sub + 1) * dh, 0:i_core]
                    nc.vector.tensor_copy(qT128[0:dh, 2 * r + sub, :], src)
                    nc.vector.tensor_copy(qT128[dh:P, 2 * r + sub, :], src)

            # qPair[k, p, m]: block-diag bias weights: rows 0-63 =
            # q_{2p} in cols 0:12, rows 64-127 = q_{2p+1} in cols 12:24.
            nc.vector.memset(qPair[:], 0.0)
            nc.vector.tensor_copy(
                qPair[0:dh, :, 0:nh],
                qT128[0:dh, :, 0::2].transpose([0, 2, 1]))
            nc.vector.tensor_copy(
                qPair[dh:P, :, nh:2 * nh],
                qT128[dh:P, :, 1::2].transpose([0, 2, 1]))

            # K projection (transposed): kT = Wk @ hid^T + bk
            WkT = load_WT("Wk", early_p.tile([P, HC, h], bf16, name="WT_k"))
            for r in range(HC):
                for jh in range(NJH):
                    pk = ps512.tile([P, JH], f32, name="big")
                    for kc in range(HC):
                        nc.tensor.matmul(pk[:], WkT[:, kc, r * P:(r + 1) * P],
                                         hidT[:, kc, jh * JH:(jh + 1) * JH],
                                         start=(kc == 0), stop=False)
                    nc.tensor.matmul(pk[:], b_bf["bk"][:, r * P:(r + 1) * P],
                                     ones_row_bf[:, 0:JH], start=False, stop=True)
                    nc.scalar.copy(kTfull[0:dh, 2 * r, jh * JH:(jh + 1) * JH],
                                   pk[0:dh, :])
                    nc.scalar.copy(kTfull[dh:P, 2 * r + 1, jh * JH:(jh + 1) * JH],
                                   pk[dh:P, :])

            # V projection (natural): v = hid @ Wv^T + bv -> v_aug[..., 0:64]
            WvT = load_WT("Wv", early_p.tile([P, HC, h], bf16, name="WT_v"))
            for jc in range(SC):
                for vh in range(2):
                    pv = ps512.tile([P, JH], f32, name="big")
                    for kc in range(HC):
                        nc.tensor.matmul(pv[:, 0:VH],
                                         hidT[:, kc, jc * P:(jc + 1) * P],
                                         WvT[:, kc, vh * VH:(vh + 1) * VH],
                                         start=(kc == 0), stop=False)
                    nc.tensor.matmul(pv[:, 0:VH], ones_row_bf[:, 0:P],
                                     b_bf["bv"][:, vh * VH:(vh + 1) * VH],
                                     start=False, stop=True)
                    nc.scalar.copy(
                        v_aug[:, jc, vh * HP:(vh + 1) * HP, 0:dh],
                        pv[:, 0:VH].rearrange("p (a b) -> p a b", a=HP))

        # ------------- halves -------------
        for half in range(2):
            i0h = half * IH
            if STAGE < 2:
                continue

            with tc.tile_pool(name="e2p", bufs=1) as e2_p:
                E2pre = e2_p.tile([P, NOCT, s], bf16)
                ctxT_all = e2_p.tile([dh, nh, P], bf16)
                if STAGE < 4:
                    nc.vector.memset(ctxT_all[:], 0.01)

                # ---- stage A: bias matmuls + fused exp -> E2pre ----
                with tc.tile_pool(name="stageA", bufs=4,
                                  space=bass.MemorySpace.PSUM) as psA:
                    for octo in range(NOCT):
                        pb_h = [psA.tile([P, JH], f32, name="pbh")
                                for j in range(NJH)]
                        for c4 in range(4):
                            pair = (i0h // 2) + octo * 4 + c4
                            iA = 2 * pair
                            bpeT = bpeT_p.tile([P, s], bf16)
                            nc.sync.dma_start(
                                bpeT[:],
                                d_bpe[iA:iA + 2].rearrange("a b c -> (a b) c"))
                            lhs = qPair[:, pair, 0:32]
                            for jh in range(NJH):
                                nc.tensor.matmul(
                                    pb_h[jh][32 * c4:32 * c4 + 32, :], lhs,
                                    bpeT[:, jh * JH:(jh + 1) * JH],
                                    tile_position=(0, 32 * c4))
                        for jh in range(NJH):
                            nc.scalar.activation(
                                E2pre[:, octo, jh * JH:(jh + 1) * JH],
                                pb_h[jh][:], Act.Exp, scale=0.125)

                # ---- stage B: transposes + QK + probs + P@V ----
                if STAGE < 3:
                    continue
                with tc.tile_pool(name="probs", bufs=2) as probs_p, \
                     tc.tile_pool(name="e1", bufs=3) as e1_p, \
                     tc.tile_pool(name="ctxp", bufs=2) as ctx_p, \
                     tc.tile_pool(name="psPV", bufs=1,
                                  space=bass.MemorySpace.PSUM) as psPV:
                    pctx = [psPV.tile([dh + 1, 4 * P], f32, name=f"pctx{g}")
                            for g in range(3)]
                    with tc.tile_pool(name="psT", bufs=1,
                                      space=bass.MemorySpace.PSUM) as psT, \
                         tc.tile_pool(name="psQK", bufs=2,
                                      space=bass.MemorySpace.PSUM) as psQK:
                        for jc in range(SC):
                            # transposes of E2pre -> ptb [j, (octo, c4, i', n)]
                            ptb = psT.tile([P, NOCT * P], bf16)
                            if SUB >= 1:
                                for o in range(NOCT):
                                    nc.tensor.matmul(
                                        ptb[:, o * P:(o + 1) * P],
                                        E2pre[:, o, jc * P:(jc + 1) * P],
                                        ident_bf[:], is_transpose=True)
                            ptb_v = ptb[:].rearrange(
                                "p (o c k) -> p o c k",
                                o=NOCT, c=4)[:, :, :, 0:2 * nh]
                            ptb_v = ptb_v.rearrange("p o c (i n) -> p o c i n",
                                                    i=2)
                            # QK + exp + mult, 4 heads at a time
                            probsT = probs_p.tile([P, nh, P], bf16)
                            if SUB < 3:
                                nc.vector.memset(probsT[:], 0.001)
                            for g in range(3):
                                if SUB < 2:
                                    break
                                pqk = psQK.tile([P, JH], f32, name="pqk")
                                for k in range(4):
                                    n = 4 * g + k
                                    nc.tensor.matmul(
                                        pqk[:, k * P:(k + 1) * P],
                                        kTfull[:, n, jc * P:(jc + 1) * P],
                                        qT128[:, n, i0h:i0h + IH])
                                e1 = e1_p.tile([P, JH], bf16)
                                nc.scalar.activation(e1[:], pqk[:], Act.Exp,
                                                     scale=0.125)
                                for k in range(4):
                                    if SUB < 3:
                                        break
                                    n = 4 * g + k
                                    nc.vector.tensor_tensor(
                                        probsT[:, n, :].rearrange(
                                            "p (o c i) -> p o c i",
                                            o=NOCT, c=4),
                                        e1[:, k * P:(k + 1) * P].rearrange(
                                            "p (o c i) -> p o c i",
                                            o=NOCT, c=4),
                                        ptb_v[:, :, :, :, n], Alu.mult)
                            # P@V (+ denominator via ones column of v_aug)
                            for n in range(nh):
                                if SUB < 4:
                                    break
                                nc.tensor.matmul(
                                    pctx[n // 4][:, (n % 4) * P:(n % 4 + 1) * P],
                                    v_aug[:, jc, n, :], probsT[:, n, :],
                                    start=(jc == 0 and n % 4 == 0),
                                    stop=(jc == SC - 1),
                                    skip_group_check=True)

                    # ---- denominators + ctx rescale ----
                    if STAGE < 4:
                        continue
                    with tc.tile_pool(name="psR", bufs=2,
                                      space=bass.MemorySpace.PSUM) as psR:
                        drec = ctx_p.tile([dh + 1, 4 * P], f32, name="drec")
                        for g in range(3):
                            nc.vector.reciprocal(drec[dh:dh + 1, :],
                                                 pctx[g][dh:dh + 1, :])
                            for k in range(4):
                                n = 4 * g + k
                                recB = psR.tile([dh, P], f32, name="recB")
                                nc.tensor.matmul(
                                    recB[:], ones_p64[dh:dh + 1, :],
                                    drec[dh:dh + 1, k * P:(k + 1) * P],
                                    tile_position=(dh, 0))
                                recBs = ctx_p.tile([dh, P], f32, name="recBs")
                                nc.scalar.copy(recBs[:], recB[:])
                                nc.vector.tensor_tensor(
                                    ctxT_all[:, n, :],
                                    pctx[g][0:dh, k * P:(k + 1) * P],
                                    recBs[:], Alu.mult)

                # ---- O-proj + residual + LN ----
                with tc.tile_pool(name="psO", bufs=2,
                                  space=bass.MemorySpace.PSUM) as psO, \
                     tc.tile_pool(name="yp", bufs=1) as y_p:
                    pys = [psO.tile([P, VH], f32, name="big") for j in range(2)]
                    for vh in range(2):
                        for n in range(nh):
                            nc.tensor.matmul(
                                pys[vh][:], ctxT_all[:, n, :],
                                WoTh[:, n, vh * VH:(vh + 1) * VH],
                                start=(n == 0), stop=False)
                        nc.tensor.matmul(pys[vh][:], ones_row_bf[:, 0:P],
                                         b_bf["bo"][:, vh * VH:(vh + 1) * VH],
                                         start=False, stop=True)
                    y = y_p.tile([P, h], f32)
                    for vh in range(2):
                        nc.vector.tensor_tensor(
                            y[:, vh * VH:(vh + 1) * VH], pys[vh][:],
                            hidR[:, half, vh * VH:(vh + 1) * VH], Alu.add)
                    mu = y_p.tile([P, 1], f32)
                    nc.vector.tensor_reduce(mu[:], y[:], AxisX, Alu.add)
                    nc.vector.tensor_scalar(mu[:], mu[:], 1.0 / h, None, Alu.mult)
                    yc = y_p.tile([P, h], f32)
                    nc.vector.tensor_scalar(yc[:], y[:], mu[:], None, Alu.subtract)
                    ssq = y_p.tile([P, 1], f32)
                    nc.scalar.activation(y[:], yc[:], Act.Square, accum_out=ssq[:])
                    std = y_p.tile([P, 1], f32)
                    nc.scalar.activation(std[:], ssq[:], Act.Sqrt,
                                         scale=1.0 / h, bias=eps_t[:])
                    rstd = y_p.tile([P, 1], f32)
                    nc.vector.reciprocal(rstd[:], std[:])
                    o1 = y_p.tile([P, h], f32)
                    nc.vector.tensor_scalar(o1[:], yc[:], rstd[:], None, Alu.mult)
                    nc.vector.tensor_tensor(o1[:], o1[:], bcast["ln_gamma"][:],
                                            Alu.mult)
                    nc.vector.tensor_tensor(o1[:], o1[:], bcast["ln_beta"][:],
                                            Alu.add)
                    nc.sync.dma_start(d_out[half], o1[:])

    nc.compile()
    return nc


def _shard_inputs(inputs):
    import ml_dtypes
    bf = ml_dtypes.bfloat16
    hs = np.ascontiguousarray(np.asarray(inputs["hidden_states"]), dtype=np.float32)
    bpe = np.asarray(inputs["bbox_pos_emb"])
    ident = np.eye(P, dtype=np.float32)
    # per-batch transposed hidden [H, S] in bf16
    hsT = {b: np.ascontiguousarray(hs[b].T.astype(bf)).reshape(H // P, P, S)
           for b in range(B)}
    WT = {w: np.ascontiguousarray(
             np.asarray(inputs[w], dtype=np.float32).T.astype(bf)).reshape(
                 H // P, P, H)
          for w in ("Wq", "Wk", "Wv")}
    # WoTh[d, n, hcol] = Wo[hcol, n*64+d]
    WoTh = np.ascontiguousarray(
        np.asarray(inputs["Wo"], dtype=np.float32).T.reshape(NH, DH, H)
        .transpose(1, 0, 2).astype(bf))
    in_maps = []
    for c in range(N_CORES):
        b = c // 4
        q0 = (c % 4) * I_CORE
        m = {
            "hidT": hsT[b],
            "hidRT": np.ascontiguousarray(
                hs[b, q0:q0 + I_CORE].T.astype(bf)).reshape(H // P, P, I_CORE),
            "hid_rows": np.ascontiguousarray(
                hs[b, q0:q0 + I_CORE].reshape(I_CORE // P, P, H)),
            "bpe": np.ascontiguousarray(
                bpe[q0:q0 + I_CORE, :, b, :].transpose(0, 2, 1).astype(bf)),
            "ident": ident,
            "WoTh": WoTh,
        }
        for w in ("Wq", "Wk", "Wv"):
            m[w + "T"] = WT[w]
        for bn in ("bq", "bk", "bv", "bo", "ln_gamma", "ln_beta"):
            m[bn] = np.ascontiguousarray(
                np.asarray(inputs[bn], dtype=np.float32).reshape(1, H))
        in_maps.append(m)
    return in_maps


def _install_ntff_shim():
    """The agent image's antenv lacks axon_hooks; recreate the NTFF profile
    hook via ctypes against libaxon_pjrt.so so trace=True yields
    exec_time_ns + a perfetto trace."""
    import sys as _sys
    if "antenv.axon_hooks" in _sys.modules:
        return
    import types, ctypes, contextlib
    so_path = "/opt/axon/libaxon_pjrt.so"
    mod = types.ModuleType("antenv.axon_hooks")
    _state = {}

    def get_axon_ntff_profile_hook():
        if "hook" in _state:
            return _state["hook"]
        try:
            lib = ctypes.CDLL(so_path)
            if not hasattr(lib, "axon_start_nrt_profile"):
                _state["hook"] = None
                return None
            lib.axon_start_nrt_profile.argtypes = [
                ctypes.POINTER(ctypes.c_int64), ctypes.c_size_t]
            lib.axon_start_nrt_profile.restype = ctypes.c_int64
            lib.axon_stop_nrt_profile.argtypes = [ctypes.c_char_p]
            lib.axon_stop_nrt_profile.restype = ctypes.c_int64
        except OSError:
            _state["hook"] = None
            return None

        @contextlib.contextmanager
        def _hook(output_dir, device_ids):
            import jax
            jax.devices()
            if device_ids:
                ids = (ctypes.c_int64 * len(device_ids))(*device_ids)
                rc = lib.axon_start_nrt_profile(ids, len(device_ids))
            else:
                rc = lib.axon_start_nrt_profile(None, 0)
            if rc != 0:
                raise RuntimeError(f"axon_start_nrt_profile rc={rc}")
            try:
                yield
            finally:
                n = lib.axon_stop_nrt_profile(str(output_dir).encode())
                print(f"ntff profile: {n} file(s) written to {output_dir}")

        _state["hook"] = _hook
        return _hook

    mod.get_axon_ntff_profile_hook = get_axon_ntff_profile_hook
    _sys.modules["antenv.axon_hooks"] = mod


def kernel(**inputs):
    from concourse.bass_utils import run_bass_kernel_spmd

    if os.environ.get("BASS_KERNEL_TRACE"):
        _install_ntff_shim()
        import concourse.bass_utils as _bu
        _bu.upload_artifacts = lambda tmpdir: f"file://{tmpdir}"

    if "nc" not in _COMPILED:
        _COMPILED["nc"] = build_kernel()
    nc = _COMPILED["nc"]
    in_maps = _shard_inputs(inputs)
    res = run_bass_kernel_spmd(nc, in_maps, core_ids=list(range(N_CORES)),
                               trace=bool(os.environ.get("BASS_KERNEL_TRACE")))
    _COMPILED["last_result"] = res
    out = np.zeros((B, S, H), dtype=np.float32)
    for c in range(N_CORES):
        b = c // 4
        q0 = (c % 4) * I_CORE
        out[b, q0:q0 + I_CORE] = np.asarray(
            res.results[c]["out"]).reshape(I_CORE, H)
    return out
